# revision 37
# baseline (speedup 1.0000x reference)
"""CMAttention Trainium2 kernel (8-core SPMD, bf16 compute).

Reference computation (per nn_CMAttention):
  q_x = (x @ Wq_x.T)  -> [b, 16, n, 64],  q_a likewise
  kv_x = x @ Wkv_x.T -> k_x, v_x [b, 1, n, 64] (single shared KV head), kv_a likewise
  l2norm + learned scales on q_x/q_a (per head) and k_x/k_a (shared)
  q = concat(q_x, q_a) [b,16,n,128]; k, v likewise [b,1,n,128]
  rotary(q, k) over the 128-dim concat axis; SDPA with softmax over keys.

Sharding: each core owns ONE batch (core//4) and FOUR heads ((core%4)*4 ..).
The shared KV projection is computed replicated on the 4 cores of a batch.

Device-side layout: everything is computed "transposed" (feature dim on
partitions, sequence on the free axis). All matmuls run in bf16 (fp32 matmul
is 4 cycles/column on TRN2; bf16 is 1), accumulating in fp32 PSUM. Softmax
runs on S^T tiles: the row-sum over keys is a ones-matmul partition reduction
accumulated in PSUM; no max subtraction is needed because q/k rows are
l2-normalized (|scores*scale| <= ~0.2 for unit scales).

Per-head rotary layout trick: the qk-norm scalar_tensor_tensor writes its
output DIRECTLY into the per-head [x-half; a-half] rotary tiles wherever the
partition ranges line up (DVE ops need matching start partitions); the
mismatched half of each tile is filled with one SBUF->SBUF DMA from its
companion tile.
"""

import numpy as np
import ml_dtypes
from contextlib import ExitStack

import concourse.bass as bass
from concourse import bacc
import concourse.mybir as mybir
import concourse.tile as tile
from concourse.masks import make_identity

F32 = mybir.dt.float32
BF16 = mybir.dt.bfloat16
AF = mybir.ActivationFunctionType
ALU = mybir.AluOpType
NPBF = ml_dtypes.bfloat16

P = 128
B, N, DIM = 2, 2048, 1024
HEADS, DH, ROT = 16, 64, 128
NCORES = 8
HPC = 4                     # heads per core (one batch per core)
KT = DIM // P               # 8 contraction tiles
SM_SCALE = float(1.0 / np.sqrt(ROT))


def build_nc(n=N, stage=0):
    CH = min(512, n)        # fp32 PSUM bank = 512 floats
    NCH = n // CH
    SU = min(1024, n)       # attention superunit width (2 PSUM banks)
    NSU = n // SU
    SUC = SU // CH
    NJT = n // P            # key tiles

    nc = bacc.Bacc()
    dp = nc.declare_dram_parameter
    xT = dp("xT", [DIM, n], BF16, isOutput=False)
    aT = dp("aT", [DIM, n], BF16, isOutput=False)
    wqx = dp("wqx", [DIM, HPC * DH], BF16, isOutput=False)
    wqa = dp("wqa", [DIM, HPC * DH], BF16, isOutput=False)
    wkvx = dp("wkvx", [DIM, P], BF16, isOutput=False)  # cols [k_x | v_x]
    wkva = dp("wkva", [DIM, P], BF16, isOutput=False)  # cols [k_a | v_a]
    sqx = dp("sqx", [P, 2], F32, isOutput=False)       # col t: heads (2t, 2t+1)
    sqa = dp("sqa", [P, 2], F32, isOutput=False)
    sk = dp("sk", [P, 2], F32, isOutput=False)         # rows 0:64: col0 kx, col1 ka
    cosT = dp("cosT", [P, n], BF16, isOutput=False)    # [cos64; cos64]
    sinT = dp("sinT", [P, n], BF16, isOutput=False)    # [-sin64; sin64]
    out = dp("out", [HPC, ROT, n], BF16, isOutput=True)

    with ExitStack() as ctx:
        tc = ctx.enter_context(tile.TileContext(nc))
        consts = ctx.enter_context(tc.tile_pool(name="consts", bufs=1))
        sb = ctx.enter_context(tc.tile_pool(name="sb", bufs=1))

        ones = consts.tile([P, P], BF16)
        nc.vector.memset(ones, 1.0)
        eps_sb = consts.tile([P, 1], F32)
        nc.vector.memset(eps_sb, 1e-24)
        ident = consts.tile([P, P], BF16)
        make_identity(nc, ident)

        sqx_sb = consts.tile([P, 2], F32)
        nc.gpsimd.dma_start(out=sqx_sb, in_=sqx[:])
        sqa_sb = consts.tile([P, 2], F32)
        nc.gpsimd.dma_start(out=sqa_sb, in_=sqa[:])
        sk_sb = consts.tile([P, 2], F32)
        nc.gpsimd.dma_start(out=sk_sb, in_=sk[:])
        cos_sb = consts.tile([P, n], BF16)
        nc.sync.dma_start(out=cos_sb, in_=cosT[:])
        sin_sb = consts.tile([P, n], BF16)
        nc.sync.dma_start(out=sin_sb, in_=sinT[:])

        w_sb = {}
        for name, hdl, m in (("wqx", wqx, HPC * DH), ("wqa", wqa, HPC * DH),
                             ("wkvx", wkvx, P), ("wkva", wkva, P)):
            w_sb[name] = consts.tile([P, KT, m], BF16, name=f"w_{name}")
        for ki in range(KT):
            for name, hdl in (("wqx", wqx), ("wqa", wqa),
                              ("wkvx", wkvx), ("wkva", wkva)):
                nc.sync.dma_start(out=w_sb[name][:, ki, :],
                                  in_=hdl[ki * P:(ki + 1) * P, :])

        # ---------------- projections ----------------
        # Per modality: Q1 (heads 0-1), Q2 (heads 2-3), KV; chunk-major so the
        # PSUM working set stays at 3 tags x 2 bufs = 6 banks.
        QT = {}   # (mod, half) -> [P, n] bf16, rows [hEven dims | hOdd dims]
        KVX = sb.tile([P, n], BF16, tag="kvx")
        KVA = sb.tile([P, n], BF16, tag="kva")
        # chunk-split input loads (c-major): chunk 0 of every k-tile lands
        # first, spread over the DMA queues, so chunk-major matmuls can start
        # after ~1/NCH of the input DMA instead of all of it
        ktiles = {}
        for mod, src in (("x", xT), ("a", aT)):
            for ki in range(KT):
                ktiles[(mod, ki)] = sb.tile([P, n], BF16, tag="ktile", bufs=10,
                                            name=f"kt_{mod}{ki}")
        for c in range(NCH):
            cs = slice(c * CH, (c + 1) * CH)
            for mod, src in (("x", xT), ("a", aT)):
                for ki in range(KT):
                    nc.sync.dma_start(out=ktiles[(mod, ki)][:, cs],
                                      in_=src[ki * P:(ki + 1) * P, cs])

        with tc.tile_pool(name="pj", bufs=1, space="PSUM") as pj:
            for mod, wq_name, wkv_name, kvdst in (
                ("x", "wqx", "wkvx", KVX), ("a", "wqa", "wkva", KVA),
            ):
                q1 = sb.tile([P, n], BF16, tag=f"q1{mod}")
                q2t = sb.tile([P, n], BF16, tag=f"q2{mod}")
                QT[(mod, 0)] = q1
                QT[(mod, 1)] = q2t
                wq_t = w_sb[wq_name]
                wkv_t = w_sb[wkv_name]
                for c in range(NCH):
                    cs = slice(c * CH, (c + 1) * CH)
                    ps1 = pj.tile([P, CH], F32, tag="p1", bufs=2)
                    ps2 = pj.tile([P, CH], F32, tag="p2", bufs=2)
                    psk = pj.tile([P, CH], F32, tag="pk", bufs=2)
                    for ki in range(KT):
                        mv = ktiles[(mod, ki)][:, cs]
                        st = (ki == 0)
                        sp = (ki == KT - 1)
                        nc.tensor.matmul(ps1, wq_t[:, ki, 0:P], mv, start=st, stop=sp)
                        nc.tensor.matmul(ps2, wq_t[:, ki, P:2 * P], mv, start=st, stop=sp)
                        nc.tensor.matmul(psk, wkv_t[:, ki, :], mv, start=st, stop=sp)
                    nc.vector.tensor_copy(q1[:, cs], ps1)
                    nc.vector.tensor_copy(q2t[:, cs], ps2)
                    nc.vector.tensor_copy(kvdst[:, cs], psk)

        # ---------------- V transpose ----------------
        # V_jt [j, d]: cols 0:64 = v_x (KVX rows 64:128), cols 64:128 = v_a
        # (KVA rows 64:128)
        V = []
        with tc.tile_pool(name="vt", bufs=2, space="PSUM") as vtp:
            for jt in range(NJT):
                js = slice(jt * P, (jt + 1) * P)
                psv1 = vtp.tile([P, DH], BF16, tag="v1")
                psv2 = vtp.tile([P, DH], BF16, tag="v2")
                nc.tensor.transpose(psv1, KVX[DH:P, js], ident[DH:P, DH:P])
                nc.tensor.transpose(psv2, KVA[DH:P, js], ident[DH:P, DH:P])
                vj = sb.tile([P, P], BF16, tag="vsb", bufs=NJT)
                nc.vector.tensor_copy(vj[:, 0:DH], psv1)
                nc.vector.tensor_copy(vj[:, DH:P], psv2)
                V.append(vj)

        # ---------------- qk-norm + per-head rotary layout ----------------
        # Per-head tiles: qh[h] rows [x-half; a-half], qsw[h] rows
        # [a-half; x-half]. The stt writes whichever target matches the source
        # partition range; the companion half is a SBUF->SBUF DMA copy.
        #
        # All streams are emitted batched by op type (squares -> sum matmuls
        # -> rsqrt -> stt -> swap DMAs) so the per-stream PE->ACT->DVE chains
        # pipeline across streams instead of serializing.
        QH = [sb.tile([P, n], BF16, tag=f"qh{h}", name=f"qh{h}") for h in range(HPC)]
        QSW = [sb.tile([P, n], BF16, tag=f"qsw{h}", name=f"qsw{h}") for h in range(HPC)]
        KH = sb.tile([P, n], BF16, tag="kh")
        KSW = sb.tile([P, n], BF16, tag="ksw")

        # units: K first (every head's attention needs krot), then heads in
        # order. Each stream: (src, r0, scale, (direct_dst, dst_r0)).
        units = [("k", KH, KSW,
                  [(KVX, 0, sk_sb[0:DH, 0:1], (KH, 0)),
                   (KVA, 0, sk_sb[0:DH, 1:2], (KSW, 0))])]
        for h in range(HPC):
            r0 = (h % 2) * DH
            col = h // 2
            if h % 2 == 0:
                ss = [(QT[("x", col)], r0,
                       sqx_sb[r0:r0 + DH, col:col + 1], (QH[h], 0)),
                      (QT[("a", col)], r0,
                       sqa_sb[r0:r0 + DH, col:col + 1], (QSW[h], 0))]
            else:
                ss = [(QT[("x", col)], r0,
                       sqx_sb[r0:r0 + DH, col:col + 1], (QSW[h], DH)),
                      (QT[("a", col)], r0,
                       sqa_sb[r0:r0 + DH, col:col + 1], (QH[h], DH))]
            units.append((f"h{h}", QH[h], QSW[h], ss))

        qrot = [None] * HPC
        krot = KH
        nm = ctx.enter_context(tc.tile_pool(name="nm", bufs=1, space="PSUM"))
        at = ctx.enter_context(tc.tile_pool(name="at", bufs=1, space="PSUM"))

        # squares hoisted: they only need the projection outputs, and keeping
        # them out of the interleaved region keeps norm matmuls unblocked
        sq_map = {}
        for uname, ht, swt, ss in units:
            for src, r0, sc, tgt in ss:
                if id(src) not in sq_map:
                    q2 = sb.tile([P, n], BF16, tag="sq", bufs=6,
                                 name=f"sq{len(sq_map)}")
                    nc.vector.tensor_mul(q2, src, src)
                    sq_map[id(src)] = q2

        def unit_ops(unit):
            """Per-chunk closures for one norm/rotary unit, to be trickled
            into the attention emission one op per key-tile iteration."""
            uname, ht, swt, ss = unit
            ops = []
            for src, r0, sc, (dst, dr0) in ss:
                r1 = r0 + DH
                q2 = sq_map[id(src)]
                for c in range(NCH):
                    def op(c=c, src=src, q2=q2, r0=r0, r1=r1, sc=sc,
                           dst=dst, dr0=dr0, uname=uname):
                        cs = slice(c * CH, (c + 1) * CH)
                        psr = nm.tile([P, CH], F32, tag="r", bufs=2,
                                      name=f"r_{uname}{c}")
                        prc = sb.tile([P, CH], F32, tag="prc", bufs=2)
                        nc.tensor.matmul(psr[r0:r1, :], ones[r0:r1, 0:DH],
                                         q2[r0:r1, cs], start=True, stop=True)
                        if r0 == 0:
                            # sqrt in PSUM, then fast approx reciprocal (only
                            # correct at base partition 0 on HW)
                            nc.scalar.activation(psr[r0:r1, :], psr[r0:r1, :],
                                                 AF.Sqrt, bias=eps_sb[r0:r1, :],
                                                 scale=1.0)
                            nc.vector.reciprocal_approx_fast(
                                out=prc[r0:r1, :], in_=psr[r0:r1, :])
                        else:
                            # rsqrt = exp(-0.5 * ln(ss)) on the scalar engine
                            nc.scalar.activation(psr[r0:r1, :], psr[r0:r1, :],
                                                 AF.Ln, bias=eps_sb[r0:r1, :],
                                                 scale=1.0)
                            nc.scalar.activation(prc[r0:r1, :], psr[r0:r1, :],
                                                 AF.Exp, bias=0.0, scale=-0.5)
                        nc.vector.scalar_tensor_tensor(
                            dst[dr0:dr0 + DH, cs], src[r0:r1, cs],
                            sc, prc[r0:r1, :], op0=ALU.mult, op1=ALU.mult)
                    ops.append(op)

            def finish(uname=uname, ht=ht, swt=swt, ss=ss):
                # companion-half swap DMAs (chunk-split across queues), then
                # rotary: rot(t) = t*cos + t_halfswap*sin_signed (sin_sb rows
                # 0:64 = -sin64, rows 64:128 = +sin64). Even units write the
                # upper halves directly; odd heads are the mirror image.
                upper_direct = ss[0][3][1] == 0
                for c in range(NCH):
                    cs = slice(c * CH, (c + 1) * CH)
                    if upper_direct:
                        nc.sync.dma_start(out=swt[DH:P, cs], in_=ht[0:DH, cs])
                        nc.sync.dma_start(out=ht[DH:P, cs], in_=swt[0:DH, cs])
                    else:
                        nc.sync.dma_start(out=ht[0:DH, cs], in_=swt[DH:P, cs])
                        nc.sync.dma_start(out=swt[0:DH, cs], in_=ht[DH:P, cs])
                tcos = sb.tile([P, n], BF16, tag="tcos", bufs=1)
                tsin = sb.tile([P, n], BF16, tag="tsin", bufs=1)
                nc.vector.tensor_mul(tcos, ht, cos_sb)
                nc.vector.tensor_mul(tsin, swt, sin_sb)
                nc.vector.tensor_add(ht, tcos, tsin)
                if uname != "k":
                    qrot[int(uname[1:])] = ht
            ops.append(finish)
            return ops

        def emit_unit(unit):
            for op in unit_ops(unit):
                op()

        if stage != 0:
            for u in units:
                emit_unit(u)

        if stage == 1:
            # dump projections + V
            nc.sync.dma_start(out=out[0], in_=QT[("x", 0)])
            nc.sync.dma_start(out=out[1], in_=QT[("a", 0)])
            nc.sync.dma_start(out=out[2], in_=KVX)
            for jt in range(NJT):
                nc.sync.dma_start(out=out[3][:, jt * P:(jt + 1) * P], in_=V[jt])
        elif stage == 2:
            # dump rotary q0/q1, krot, V
            nc.sync.dma_start(out=out[0], in_=qrot[0])
            nc.sync.dma_start(out=out[1], in_=qrot[1])
            nc.sync.dma_start(out=out[2], in_=krot)
            for jt in range(NJT):
                nc.sync.dma_start(out=out[3][:, jt * P:(jt + 1) * P], in_=V[jt])

        # ---------------- attention ----------------
        def emit_scores(h, su, jt):
            js = slice(jt * P, (jt + 1) * P)
            ps_s = at.tile([P, SU], F32, tag="s", bufs=2, name=f"s{h}_{su}_{jt}")
            for cc in range(SUC):
                el = slice(cc * CH, (cc + 1) * CH)
                il = slice(su * SU + cc * CH, su * SU + (cc + 1) * CH)
                nc.tensor.matmul(ps_s[:, el], krot[:, js], qrot[h][:, il],
                                 start=True, stop=True)
            return ps_s

        # software pipeline: scores(jt+1) is emitted (PE queue) before the
        # exp-dependent AV matmuls of jt, so the PE never waits on the
        # scalar engine's exp round-trip. The softmax denominator is
        # accumulated across key-tiles on the vector engine (bf16 adds)
        # and partition-reduced with a single ones-matmul at the end,
        # instead of a per-key-tile ones-matmul on the PE.
        def emit_attn(h, su, feeder=None):
            ps_o = at.tile([P, SU], F32, tag="o", bufs=1, name=f"o{h}_{su}")
            ps_s = emit_scores(h, su, 0)
            acc = None
            for jt in range(NJT):
                es = sb.tile([P, SU], BF16, tag="es", bufs=3)
                nc.scalar.activation(es, ps_s, AF.Exp, bias=0.0,
                                     scale=SM_SCALE)
                if jt + 1 < NJT:
                    ps_s = emit_scores(h, su, jt + 1)
                for cc in range(SUC):
                    el = slice(cc * CH, (cc + 1) * CH)
                    nc.tensor.matmul(ps_o[:, el], V[jt], es[:, el],
                                     start=(jt == 0), stop=(jt == NJT - 1))
                if acc is None:
                    acc = es
                else:
                    nacc = sb.tile([P, SU], BF16, tag="acc", bufs=2)
                    nc.vector.tensor_add(nacc, acc, es)
                    acc = nacc
                if feeder:
                    feeder.popleft()[1]()
            ps_den = at.tile([P, SU], F32, tag="s", bufs=2, name=f"d{h}_{su}")
            for cc in range(SUC):
                el = slice(cc * CH, (cc + 1) * CH)
                nc.tensor.matmul(ps_den[:, el], ones, acc[:, el],
                                 start=True, stop=True)
            rec = sb.tile([P, SU], F32, tag="rec", bufs=2)
            nc.vector.reciprocal_approx_fast(out=rec, in_=ps_den)
            on = sb.tile([P, SU], BF16, tag="on", bufs=2)
            nc.vector.tensor_mul(on, ps_o, rec)
            for cc in range(SUC):
                el = slice(cc * CH, (cc + 1) * CH)
                nc.sync.dma_start(
                    out=out[h, :, su * SU + cc * CH:su * SU + (cc + 1) * CH],
                    in_=on[:, el])

        if stage == 0:
            # interleave: the remaining norm/rotary units trickle into the
            # attention emission one chunk-op per key-tile iteration, so the
            # ACT/DVE norm chains overlap attention without ever blocking
            # the in-order PE queue.
            from collections import deque
            emit_unit(units[0])   # K (krot needed by every head)
            emit_unit(units[1])   # h0
            pending = deque()     # (unit_idx, op)
            next_u = 2
            for h in range(HPC):
                for su in range(NSU):
                    while next_u <= h + 1:    # unit for head h is units[h+1]
                        pending.extend(
                            (next_u, op) for op in unit_ops(units[next_u]))
                        next_u += 1
                    while pending and pending[0][0] <= h + 1:
                        pending.popleft()[1]()
                    if not pending and next_u < len(units):
                        pending.extend(
                            (next_u, op) for op in unit_ops(units[next_u]))
                        next_u += 1
                    emit_attn(h, su, feeder=pending)
    nc.finalize()
    return nc


# ---------------------------------------------------------------------------
# host side
# ---------------------------------------------------------------------------

_NC_CACHE = {}


def get_nc(n=N, nb=B):
    key = n
    if key not in _NC_CACHE:
        _NC_CACHE[key] = build_nc(n)
    return _NC_CACHE[key]


def rotary_tables(n):
    inv_freq = 1.0 / (10000.0 ** (np.arange(0, ROT, 2, dtype=np.float64) / ROT))
    freqs = np.outer(np.arange(n, dtype=np.float64), inv_freq)  # [n, 64]
    cos64 = np.cos(freqs).T.astype(np.float32)                  # [64, n]
    sin64 = np.sin(freqs).T.astype(np.float32)
    cosT = np.ascontiguousarray(np.concatenate([cos64, cos64], 0)).astype(NPBF)
    sinT = np.ascontiguousarray(np.concatenate([-sin64, sin64], 0)).astype(NPBF)
    return cosT, sinT


def prep_in_maps(inputs, n=N, nb=B, ncores=NCORES):
    g = {k: np.asarray(v, dtype=np.float32) for k, v in inputs.items()}
    xT = [np.ascontiguousarray(g["x"][b].T).astype(NPBF) for b in range(nb)]
    aT = [np.ascontiguousarray(g["a"][b].T).astype(NPBF) for b in range(nb)]
    wkvx = np.ascontiguousarray(g["Wkv_x"].T).astype(NPBF)          # cols [kx|vx]
    wkva = np.ascontiguousarray(g["Wkv_a"].T).astype(NPBF)          # cols [ka|va]
    sk = np.zeros((P, 2), np.float32)                               # rows 0:64 only
    sk[0:DH, 0] = g["kx_scale"][0, 0]
    sk[0:DH, 1] = g["ka_scale"][0, 0]
    cosT, sinT = rotary_tables(n)

    in_maps = []
    for c in range(ncores):
        b = c // (ncores // nb)
        h0 = (c % (ncores // nb)) * HPC
        m = dict(xT=xT[b], aT=aT[b], wkvx=wkvx, wkva=wkva, sk=sk,
                 cosT=cosT, sinT=sinT)
        m["wqx"] = np.ascontiguousarray(
            g["Wq_x"][h0 * DH:(h0 + HPC) * DH].T).astype(NPBF)
        m["wqa"] = np.ascontiguousarray(
            g["Wq_a"][h0 * DH:(h0 + HPC) * DH].T).astype(NPBF)
        m["sqx"] = np.ascontiguousarray(np.stack(
            [np.concatenate([g["qx_scale"][h0 + 2 * t, 0],
                             g["qx_scale"][h0 + 2 * t + 1, 0]]) for t in range(2)],
            axis=1)).astype(np.float32)
        m["sqa"] = np.ascontiguousarray(np.stack(
            [np.concatenate([g["qa_scale"][h0 + 2 * t, 0],
                             g["qa_scale"][h0 + 2 * t + 1, 0]]) for t in range(2)],
            axis=1)).astype(np.float32)
        in_maps.append(m)
    return in_maps


def gather_out(results, n=N, nb=B, ncores=NCORES):
    full = np.empty((nb, n, HEADS * ROT), np.float32)
    for c in range(ncores):
        b = c // (ncores // nb)
        h0 = (c % (ncores // nb)) * HPC
        o = np.asarray(results[c]["out"]).astype(np.float32)  # [HPC, ROT, n]
        for h in range(HPC):
            gh = h0 + h
            full[b, :, gh * ROT:(gh + 1) * ROT] = o[h].T
    return full


def kernel(**inputs):
    from concourse.bass_utils import run_bass_kernel_spmd
    nc = get_nc(N, B)
    in_maps = prep_in_maps(inputs, N, B, NCORES)
    res = run_bass_kernel_spmd(nc, in_maps, list(range(NCORES)))
    return gather_out(res.results, N, B, NCORES)


if __name__ == "__main__":
    build_nc(256)
    print("build ok")


# revision 50
# speedup vs baseline: 1.0729x; 1.0729x over previous
"""CMAttention Trainium2 kernel (8-core SPMD, bf16 compute).

Reference computation (per nn_CMAttention):
  q_x = (x @ Wq_x.T)  -> [b, 16, n, 64],  q_a likewise
  kv_x = x @ Wkv_x.T -> k_x, v_x [b, 1, n, 64] (single shared KV head), kv_a likewise
  l2norm + learned scales on q_x/q_a (per head) and k_x/k_a (shared)
  q = concat(q_x, q_a) [b,16,n,128]; k, v likewise [b,1,n,128]
  rotary(q, k) over the 128-dim concat axis; SDPA with softmax over keys.

Sharding: each core owns ONE batch (core//4) and FOUR heads ((core%4)*4 ..).
The shared KV projection is computed replicated on the 4 cores of a batch.

Device-side layout: everything is computed "transposed" (feature dim on
partitions, sequence on the free axis). All matmuls run in bf16 (fp32 matmul
is 4 cycles/column on TRN2; bf16 is 1), accumulating in fp32 PSUM. Softmax
runs on S^T tiles: the row-sum over keys is a ones-matmul partition reduction
accumulated in PSUM; no max subtraction is needed because q/k rows are
l2-normalized (|scores*scale| <= ~0.2 for unit scales).

Per-head rotary layout trick: the qk-norm scalar_tensor_tensor writes its
output DIRECTLY into the per-head [x-half; a-half] rotary tiles wherever the
partition ranges line up (DVE ops need matching start partitions); the
mismatched half of each tile is filled with one SBUF->SBUF DMA from its
companion tile.
"""

import numpy as np
import ml_dtypes
from contextlib import ExitStack

import concourse.bass as bass
from concourse import bacc
import concourse.mybir as mybir
import concourse.tile as tile
from concourse.masks import make_identity

F32 = mybir.dt.float32
BF16 = mybir.dt.bfloat16
AF = mybir.ActivationFunctionType
ALU = mybir.AluOpType
NPBF = ml_dtypes.bfloat16

P = 128
B, N, DIM = 2, 2048, 1024
HEADS, DH, ROT = 16, 64, 128
NCORES = 8
HPC = 4                     # heads per core (one batch per core)
KT = DIM // P               # 8 contraction tiles
SM_SCALE = float(1.0 / np.sqrt(ROT))


def build_nc(n=N, stage=0):
    CH = min(512, n)        # fp32 PSUM bank = 512 floats
    NCH = n // CH
    SU = min(1024, n)       # attention superunit width (2 PSUM banks)
    NSU = n // SU
    SUC = SU // CH
    NJT = n // P            # key tiles

    nc = bacc.Bacc()
    dp = nc.declare_dram_parameter
    xT = dp("xT", [DIM, n], BF16, isOutput=False)
    aT = dp("aT", [DIM, n], BF16, isOutput=False)
    wqx = dp("wqx", [DIM, HPC * DH], BF16, isOutput=False)
    wqa = dp("wqa", [DIM, HPC * DH], BF16, isOutput=False)
    wkvx = dp("wkvx", [DIM, P], BF16, isOutput=False)  # cols [k_x | v_x]
    wkva = dp("wkva", [DIM, P], BF16, isOutput=False)  # cols [k_a | v_a]
    sqx = dp("sqx", [P, 2], F32, isOutput=False)       # col t: heads (2t, 2t+1)
    sqa = dp("sqa", [P, 2], F32, isOutput=False)
    sk = dp("sk", [P, 2], F32, isOutput=False)         # rows 0:64: col0 kx, col1 ka
    cosT = dp("cosT", [P, n], BF16, isOutput=False)    # [cos64; cos64]
    sinT = dp("sinT", [P, n], BF16, isOutput=False)    # [-sin64; sin64]
    out = dp("out", [HPC, ROT, n], BF16, isOutput=True)

    with ExitStack() as ctx:
        tc = ctx.enter_context(tile.TileContext(nc))
        consts = ctx.enter_context(tc.tile_pool(name="consts", bufs=1))
        sb = ctx.enter_context(tc.tile_pool(name="sb", bufs=1))

        ones = consts.tile([P, P], BF16)
        nc.vector.memset(ones, 1.0)
        eps_sb = consts.tile([P, 1], F32)
        nc.vector.memset(eps_sb, 1e-24)
        ident = consts.tile([P, P], BF16)
        make_identity(nc, ident)

        sqx_sb = consts.tile([P, 2], F32)
        nc.gpsimd.dma_start(out=sqx_sb, in_=sqx[:])
        sqa_sb = consts.tile([P, 2], F32)
        nc.gpsimd.dma_start(out=sqa_sb, in_=sqa[:])
        sk_sb = consts.tile([P, 2], F32)
        nc.gpsimd.dma_start(out=sk_sb, in_=sk[:])
        cos_sb = consts.tile([P, n], BF16)
        nc.sync.dma_start(out=cos_sb, in_=cosT[:])
        sin_sb = consts.tile([P, n], BF16)
        nc.sync.dma_start(out=sin_sb, in_=sinT[:])

        w_sb = {}
        for name, hdl, m in (("wqx", wqx, HPC * DH), ("wqa", wqa, HPC * DH),
                             ("wkvx", wkvx, P), ("wkva", wkva, P)):
            w_sb[name] = consts.tile([P, KT, m], BF16, name=f"w_{name}")
        for ki in range(KT):
            for name, hdl in (("wqx", wqx), ("wqa", wqa),
                              ("wkvx", wkvx), ("wkva", wkva)):
                nc.sync.dma_start(out=w_sb[name][:, ki, :],
                                  in_=hdl[ki * P:(ki + 1) * P, :])

        # ---------------- projections ----------------
        # Per modality: Q1 (heads 0-1), Q2 (heads 2-3), KV; chunk-major so the
        # PSUM working set stays at 3 tags x 2 bufs = 6 banks.
        QT = {(mod, half): sb.tile([P, n], BF16, tag=f"q{half}{mod}",
                                   name=f"qt_{mod}{half}")
              for mod in ("x", "a") for half in (0, 1)}
        # (mod, half) -> [P, n] bf16, rows [hEven dims | hOdd dims]
        KVX = sb.tile([P, n], BF16, tag="kvx")
        KVA = sb.tile([P, n], BF16, tag="kva")
        # chunk-split input loads (c-major): chunk 0 of every k-tile lands
        # first, spread over the DMA queues, so chunk-major matmuls can start
        # after ~1/NCH of the input DMA instead of all of it
        ktiles = {}
        for mod, src in (("x", xT), ("a", aT)):
            for ki in range(KT):
                ktiles[(mod, ki)] = sb.tile([P, n], BF16, tag="ktile", bufs=10,
                                            name=f"kt_{mod}{ki}")
        for c in range(NCH):
            cs = slice(c * CH, (c + 1) * CH)
            for mod, src in (("x", xT), ("a", aT)):
                for ki in range(KT):
                    nc.sync.dma_start(out=ktiles[(mod, ki)][:, cs],
                                      in_=src[ki * P:(ki + 1) * P, cs])

        def emit_proj(mod, wq_name, wkv_name, kvdst, pj, feeder=None):
            # chunk-PAIR major: each stationary weight tile issues two
            # back-to-back matmuls (both chunks of the pair) so LDWEIGHTS
            # count halves and loads hide under the previous matmul
            q1 = QT[(mod, 0)]
            q2t = QT[(mod, 1)]
            wq_t = w_sb[wq_name]
            wkv_t = w_sb[wkv_name]
            for cp in range(NCH // 2):
                pps = [[pj.tile([P, CH], F32, tag=f"p{t}{cc}", bufs=1,
                                name=f"pp_{mod}{cp}_{t}{cc}")
                        for cc in range(2)] for t in range(3)]
                for ki in range(KT):
                    st = (ki == 0)
                    sp = (ki == KT - 1)
                    for t, wsl in ((0, wq_t[:, ki, 0:P]),
                                   (1, wq_t[:, ki, P:2 * P]),
                                   (2, wkv_t[:, ki, :])):
                        for cc in range(2):
                            mv = ktiles[(mod, ki)][:, (2 * cp + cc) * CH:
                                                   (2 * cp + cc + 1) * CH]
                            nc.tensor.matmul(pps[t][cc], wsl, mv,
                                             start=st, stop=sp)
                for cc in range(2):
                    cs = slice((2 * cp + cc) * CH, (2 * cp + cc + 1) * CH)
                    nc.vector.tensor_copy(q1[:, cs], pps[0][cc])
                    nc.vector.tensor_copy(q2t[:, cs], pps[1][cc])
                    nc.vector.tensor_copy(kvdst[:, cs], pps[2][cc])
                if feeder:
                    for _ in range(min(10, len(feeder))):
                        feeder.popleft()[1]()

        # ---------------- V transpose ----------------
        # V_jt [j, d]: cols 0:64 = v_x (KVX rows 64:128), cols 64:128 = v_a
        # (KVA rows 64:128)
        V = []

        def emit_vt():
            with tc.tile_pool(name="vt", bufs=1, space="PSUM") as vtp:
                for jt in range(NJT):
                    js = slice(jt * P, (jt + 1) * P)
                    psv1 = vtp.tile([P, DH], BF16, tag="v1")
                    psv2 = vtp.tile([P, DH], BF16, tag="v2")
                    nc.tensor.transpose(psv1, KVX[DH:P, js], ident[DH:P, DH:P])
                    nc.tensor.transpose(psv2, KVA[DH:P, js], ident[DH:P, DH:P])
                    vj = sb.tile([P, P], BF16, tag="vsb", bufs=NJT)
                    nc.vector.tensor_copy(vj[:, 0:DH], psv1)
                    nc.vector.tensor_copy(vj[:, DH:P], psv2)
                    V.append(vj)

        # ---------------- qk-norm + per-head rotary layout ----------------
        # Per-head tiles: qh[h] rows [x-half; a-half], qsw[h] rows
        # [a-half; x-half]. The stt writes whichever target matches the source
        # partition range; the companion half is a SBUF->SBUF DMA copy.
        #
        # All streams are emitted batched by op type (squares -> sum matmuls
        # -> rsqrt -> stt -> swap DMAs) so the per-stream PE->ACT->DVE chains
        # pipeline across streams instead of serializing.
        QH = [sb.tile([P, n], BF16, tag=f"qh{h}", name=f"qh{h}") for h in range(HPC)]
        QSW = [sb.tile([P, n], BF16, tag=f"qsw{h}", name=f"qsw{h}") for h in range(HPC)]
        KH = sb.tile([P, n], BF16, tag="kh")
        KSW = sb.tile([P, n], BF16, tag="ksw")

        # units: K first (every head's attention needs krot), then heads in
        # order. Each stream: (src, r0, scale, (direct_dst, dst_r0)).
        units = [("k", KH, KSW,
                  [(KVX, 0, sk_sb[0:DH, 0:1], (KH, 0)),
                   (KVA, 0, sk_sb[0:DH, 1:2], (KSW, 0))])]
        for h in range(HPC):
            r0 = (h % 2) * DH
            col = h // 2
            if h % 2 == 0:
                ss = [(QT[("x", col)], r0,
                       sqx_sb[r0:r0 + DH, col:col + 1], (QH[h], 0)),
                      (QT[("a", col)], r0,
                       sqa_sb[r0:r0 + DH, col:col + 1], (QSW[h], 0))]
            else:
                ss = [(QT[("x", col)], r0,
                       sqx_sb[r0:r0 + DH, col:col + 1], (QSW[h], DH)),
                      (QT[("a", col)], r0,
                       sqa_sb[r0:r0 + DH, col:col + 1], (QH[h], DH))]
            units.append((f"h{h}", QH[h], QSW[h], ss))

        qrot = [None] * HPC
        krot = KH
        nm = ctx.enter_context(tc.tile_pool(name="nm", bufs=1, space="PSUM"))
        at = None   # attention PSUM pool, opened after the projection pool closes

        sq_map = {}

        def emit_squares(srcs):
            for src in srcs:
                if id(src) not in sq_map:
                    q2 = sb.tile([P, n], BF16, tag="sq", bufs=6,
                                 name=f"sq{len(sq_map)}")
                    nc.vector.tensor_mul(q2, src, src)
                    sq_map[id(src)] = q2

        def stream_ops(uname, src, r0, sc, dst, dr0):
            """Per-chunk closures (mm -> rsqrt -> stt) for one norm stream."""
            r1 = r0 + DH
            ops = []
            for c in range(NCH):
                def op(c=c):
                    q2 = sq_map[id(src)]
                    cs = slice(c * CH, (c + 1) * CH)
                    psr = nm.tile([P, CH], F32, tag="r", bufs=2,
                                  name=f"r_{uname}{c}")
                    prc = sb.tile([P, CH], F32, tag="prc", bufs=2)
                    nc.tensor.matmul(psr[r0:r1, :], ones[r0:r1, 0:DH],
                                     q2[r0:r1, cs], start=True, stop=True)
                    if r0 == 0:
                        # sqrt in PSUM, then fast approx reciprocal (only
                        # correct at base partition 0 on HW)
                        nc.scalar.activation(psr[r0:r1, :], psr[r0:r1, :],
                                             AF.Sqrt, bias=eps_sb[r0:r1, :],
                                             scale=1.0)
                        nc.vector.reciprocal_approx_fast(
                            out=prc[r0:r1, :], in_=psr[r0:r1, :])
                    else:
                        # rsqrt = exp(-0.5 * ln(ss)) on the scalar engine
                        nc.scalar.activation(psr[r0:r1, :], psr[r0:r1, :],
                                             AF.Ln, bias=eps_sb[r0:r1, :],
                                             scale=1.0)
                        nc.scalar.activation(prc[r0:r1, :], psr[r0:r1, :],
                                             AF.Exp, bias=0.0, scale=-0.5)
                    nc.vector.scalar_tensor_tensor(
                        dst[dr0:dr0 + DH, cs], src[r0:r1, cs],
                        sc, prc[r0:r1, :], op0=ALU.mult, op1=ALU.mult)
                ops.append(op)
            return ops

        def unit_finish(unit):
            # companion-half swap DMAs (chunk-split across queues), then
            # rotary: rot(t) = t*cos + t_halfswap*sin_signed (sin_sb rows
            # 0:64 = -sin64, rows 64:128 = +sin64). Even units write the
            # upper halves directly; odd heads are the mirror image.
            uname, ht, swt, ss = unit
            upper_direct = ss[0][3][1] == 0
            for c in range(NCH):
                cs = slice(c * CH, (c + 1) * CH)
                if upper_direct:
                    nc.sync.dma_start(out=swt[DH:P, cs], in_=ht[0:DH, cs])
                    nc.sync.dma_start(out=ht[DH:P, cs], in_=swt[0:DH, cs])
                else:
                    nc.sync.dma_start(out=ht[0:DH, cs], in_=swt[DH:P, cs])
                    nc.sync.dma_start(out=swt[0:DH, cs], in_=ht[DH:P, cs])
            tcos = sb.tile([P, n], BF16, tag="tcos", bufs=1)
            tsin = sb.tile([P, n], BF16, tag="tsin", bufs=1)
            nc.vector.tensor_mul(tcos, ht, cos_sb)
            nc.vector.tensor_mul(tsin, swt, sin_sb)
            nc.vector.tensor_add(ht, tcos, tsin)
            if uname != "k":
                qrot[int(uname[1:])] = ht

        def unit_x_ops(unit):
            uname, ht, swt, ss = unit
            src, r0, sc, (dst, dr0) = ss[0]
            return stream_ops(uname + "x", src, r0, sc, dst, dr0)

        def unit_a_ops(unit):
            uname, ht, swt, ss = unit
            src, r0, sc, (dst, dr0) = ss[1]
            return stream_ops(uname + "a", src, r0, sc, dst, dr0)

        def emit_unit(unit):
            emit_squares([s[0] for s in unit[3]])
            for op in unit_x_ops(unit) + unit_a_ops(unit):
                op()
            unit_finish(unit)

        if stage != 0:
            with tc.tile_pool(name="pj", bufs=1, space="PSUM") as pj:
                emit_proj("x", "wqx", "wkvx", KVX, pj)
                emit_proj("a", "wqa", "wkva", KVA, pj)
            emit_vt()
            for u in units:
                emit_unit(u)

        if stage == 1:
            # dump projections + V
            nc.sync.dma_start(out=out[0], in_=QT[("x", 0)])
            nc.sync.dma_start(out=out[1], in_=QT[("a", 0)])
            nc.sync.dma_start(out=out[2], in_=KVX)
            for jt in range(NJT):
                nc.sync.dma_start(out=out[3][:, jt * P:(jt + 1) * P], in_=V[jt])
        elif stage == 2:
            # dump rotary q0/q1, krot, V
            nc.sync.dma_start(out=out[0], in_=qrot[0])
            nc.sync.dma_start(out=out[1], in_=qrot[1])
            nc.sync.dma_start(out=out[2], in_=krot)
            for jt in range(NJT):
                nc.sync.dma_start(out=out[3][:, jt * P:(jt + 1) * P], in_=V[jt])

        # ---------------- attention ----------------
        def emit_scores(h, su, jt):
            js = slice(jt * P, (jt + 1) * P)
            ps_s = at.tile([P, SU], F32, tag="s", bufs=2, name=f"s{h}_{su}_{jt}")
            for cc in range(SUC):
                el = slice(cc * CH, (cc + 1) * CH)
                il = slice(su * SU + cc * CH, su * SU + (cc + 1) * CH)
                nc.tensor.matmul(ps_s[:, el], krot[:, js], qrot[h][:, il],
                                 start=True, stop=True)
            return ps_s

        # software pipeline: scores(jt+1) is emitted (PE queue) before the
        # exp-dependent AV matmuls of jt, so the PE never waits on the
        # scalar engine's exp round-trip. The softmax denominator is
        # accumulated across key-tiles on the vector engine (bf16 adds)
        # and partition-reduced with a single ones-matmul at the end,
        # instead of a per-key-tile ones-matmul on the PE.
        def emit_attn(h, su, feeder=None):
            ps_o = at.tile([P, SU], F32, tag="o", bufs=1, name=f"o{h}_{su}")
            ps_s = emit_scores(h, su, 0)
            acc = None
            for jt in range(NJT):
                es = sb.tile([P, SU], BF16, tag="es", bufs=3)
                nc.scalar.activation(es, ps_s, AF.Exp, bias=0.0,
                                     scale=SM_SCALE)
                if jt + 1 < NJT:
                    ps_s = emit_scores(h, su, jt + 1)
                for cc in range(SUC):
                    el = slice(cc * CH, (cc + 1) * CH)
                    nc.tensor.matmul(ps_o[:, el], V[jt], es[:, el],
                                     start=(jt == 0), stop=(jt == NJT - 1))
                if acc is None:
                    acc = es
                else:
                    nacc = sb.tile([P, SU], BF16, tag="acc", bufs=2)
                    nc.vector.tensor_add(nacc, acc, es)
                    acc = nacc
                if feeder:
                    feeder.popleft()[1]()
            ps_den = at.tile([P, SU], F32, tag="s", bufs=2, name=f"d{h}_{su}")
            for cc in range(SUC):
                el = slice(cc * CH, (cc + 1) * CH)
                nc.tensor.matmul(ps_den[:, el], ones, acc[:, el],
                                 start=True, stop=True)
            rec = sb.tile([P, SU], F32, tag="rec", bufs=2)
            nc.vector.reciprocal_approx_fast(out=rec, in_=ps_den)
            on = sb.tile([P, SU], BF16, tag="on", bufs=2)
            nc.vector.tensor_mul(on, ps_o, rec)
            for cc in range(SUC):
                el = slice(cc * CH, (cc + 1) * CH)
                nc.sync.dma_start(
                    out=out[h, :, su * SU + cc * CH:su * SU + (cc + 1) * CH],
                    in_=on[:, el])

        if stage == 0:
            # Schedule: x-modality projection; then the x-side norm chains
            # trickle into the a-modality projection (whose PE work hides
            # their ACT/DVE latency); V transposes; then a-side norm chains
            # + per-unit rotary finish; then attention, which is cleanly
            # ACT(exp)-bound with the PE ~90% busy.
            from collections import deque
            with tc.tile_pool(name="pj", bufs=1, space="PSUM") as pj:
                emit_proj("x", "wqx", "wkvx", KVX, pj)
                emit_squares([KVX, QT[("x", 0)], QT[("x", 1)]])
                xfeed = deque(
                    (0, op) for u in units for op in unit_x_ops(u))
                emit_proj("a", "wqa", "wkva", KVA, pj, feeder=xfeed)
            while xfeed:
                xfeed.popleft()[1]()
            emit_vt()
            at = ctx.enter_context(tc.tile_pool(name="at", bufs=1, space="PSUM"))
            emit_squares([KVA, QT[("a", 0)], QT[("a", 1)]])
            for u in units:
                for op in unit_a_ops(u):
                    op()
                unit_finish(u)
            for h in range(HPC):
                for su in range(NSU):
                    emit_attn(h, su)
    nc.finalize()
    return nc


# ---------------------------------------------------------------------------
# host side
# ---------------------------------------------------------------------------

_NC_CACHE = {}


def get_nc(n=N, nb=B):
    key = n
    if key not in _NC_CACHE:
        _NC_CACHE[key] = build_nc(n)
    return _NC_CACHE[key]


def rotary_tables(n):
    inv_freq = 1.0 / (10000.0 ** (np.arange(0, ROT, 2, dtype=np.float64) / ROT))
    freqs = np.outer(np.arange(n, dtype=np.float64), inv_freq)  # [n, 64]
    cos64 = np.cos(freqs).T.astype(np.float32)                  # [64, n]
    sin64 = np.sin(freqs).T.astype(np.float32)
    cosT = np.ascontiguousarray(np.concatenate([cos64, cos64], 0)).astype(NPBF)
    sinT = np.ascontiguousarray(np.concatenate([-sin64, sin64], 0)).astype(NPBF)
    return cosT, sinT


def prep_in_maps(inputs, n=N, nb=B, ncores=NCORES):
    g = {k: np.asarray(v, dtype=np.float32) for k, v in inputs.items()}
    xT = [np.ascontiguousarray(g["x"][b].T).astype(NPBF) for b in range(nb)]
    aT = [np.ascontiguousarray(g["a"][b].T).astype(NPBF) for b in range(nb)]
    wkvx = np.ascontiguousarray(g["Wkv_x"].T).astype(NPBF)          # cols [kx|vx]
    wkva = np.ascontiguousarray(g["Wkv_a"].T).astype(NPBF)          # cols [ka|va]
    sk = np.zeros((P, 2), np.float32)                               # rows 0:64 only
    sk[0:DH, 0] = g["kx_scale"][0, 0]
    sk[0:DH, 1] = g["ka_scale"][0, 0]
    cosT, sinT = rotary_tables(n)

    in_maps = []
    for c in range(ncores):
        b = c // (ncores // nb)
        h0 = (c % (ncores // nb)) * HPC
        m = dict(xT=xT[b], aT=aT[b], wkvx=wkvx, wkva=wkva, sk=sk,
                 cosT=cosT, sinT=sinT)
        m["wqx"] = np.ascontiguousarray(
            g["Wq_x"][h0 * DH:(h0 + HPC) * DH].T).astype(NPBF)
        m["wqa"] = np.ascontiguousarray(
            g["Wq_a"][h0 * DH:(h0 + HPC) * DH].T).astype(NPBF)
        m["sqx"] = np.ascontiguousarray(np.stack(
            [np.concatenate([g["qx_scale"][h0 + 2 * t, 0],
                             g["qx_scale"][h0 + 2 * t + 1, 0]]) for t in range(2)],
            axis=1)).astype(np.float32)
        m["sqa"] = np.ascontiguousarray(np.stack(
            [np.concatenate([g["qa_scale"][h0 + 2 * t, 0],
                             g["qa_scale"][h0 + 2 * t + 1, 0]]) for t in range(2)],
            axis=1)).astype(np.float32)
        in_maps.append(m)
    return in_maps


def gather_out(results, n=N, nb=B, ncores=NCORES):
    full = np.empty((nb, n, HEADS * ROT), np.float32)
    for c in range(ncores):
        b = c // (ncores // nb)
        h0 = (c % (ncores // nb)) * HPC
        o = np.asarray(results[c]["out"]).astype(np.float32)  # [HPC, ROT, n]
        for h in range(HPC):
            gh = h0 + h
            full[b, :, gh * ROT:(gh + 1) * ROT] = o[h].T
    return full


def kernel(**inputs):
    from concourse.bass_utils import run_bass_kernel_spmd
    nc = get_nc(N, B)
    in_maps = prep_in_maps(inputs, N, B, NCORES)
    res = run_bass_kernel_spmd(nc, in_maps, list(range(NCORES)))
    return gather_out(res.results, N, B, NCORES)


if __name__ == "__main__":
    build_nc(256)
    print("build ok")
